# revision 7
# baseline (speedup 1.0000x reference)
"""Multi-head attention (B=2, S=2048, D=2048, H=16, hd=128) on 8 TRN2 NeuronCores.

Sharding: data-parallel over batch (2) x tensor-parallel over head groups (4).
Core c handles batch c//4 and heads [4*(c%4), 4*(c%4)+4). Each core computes
q/k/v projections for its 512 features, RoPE, full attention over S for its 4
heads, and a partial output projection y_partial = attn_local @ wo[:, cols].T.
Host sums the 4 fp16 partials per batch (no on-chip collectives).

All matmuls run in f16 with fp32 PSUM accumulation. The 1/sqrt(hd) score
scale is folded into wq host-side. RoPE pairs are split even/odd across the
partition dim by permuting wq/wk rows host-side, so RoPE is elementwise DVE
work against stacked [cos;cos] / [sin;sin] tables. Scores are computed
transposed ([k, q]) so softmax(exp)@V needs no on-chip transposes.

The softmax denominator's cross-partition reduction is a single matmul
against an all-ones stationary tile (broadcasting the column sums to all 128
partitions), so the exp-sum -> reciprocal -> divide chain is short and never
stalls the PSUM rings. One PSUM pool (tags pvps/pyps/ss) spans all phases so
there are no pool-transition barriers. Initial weight/x DMAs are issued in
dc-quarter slices so the first projection matmul gates on ~1MB of HBM
traffic instead of the full tiles. x is streamed twice (k pass, then a
combined q+v pass that reuses each chunk tile for both projections).
"""

import numpy as np

B = 2
S = 2048
D = 2048
H = 16
HD = 128
P = 128
N_CORES = 8
H_LOC = 4          # heads per core
F = H_LOC * HD     # local features = 512
NCH = 4            # n-chunks of 512 over S
CH = S // NCH      # 512
DCH = D // P       # 16 contraction chunks
NT = S // P        # 16 row tiles

_F16 = np.float16


def _build_program():
    import concourse.mybir as mybir
    import concourse.tile as tile
    from concourse import bacc

    dt = mybir.dt
    nc = bacc.Bacc("TRN2", target_bir_lowering=False, debug=False,
                   num_devices=N_CORES)

    # partition-major layouts so every DMA reads >=2KB contiguous per line
    xTc = nc.dram_tensor("xTc", [NCH, P, DCH, CH], dt.float16,
                         kind="ExternalInput").ap()
    wqT = nc.dram_tensor("wqT", [P, DCH, F], dt.float16,
                         kind="ExternalInput").ap()
    wkT = nc.dram_tensor("wkT", [P, DCH, F], dt.float16,
                         kind="ExternalInput").ap()
    wvT = nc.dram_tensor("wvT", [P, DCH, F], dt.float16,
                         kind="ExternalInput").ap()
    woT = nc.dram_tensor("woT", [P, H_LOC, D], dt.float16,
                         kind="ExternalInput").ap()
    # stacked RoPE tables: [cos;cos] and [sin;sin]
    ct = nc.dram_tensor("ct", [P, S], dt.float16, kind="ExternalInput").ap()
    st = nc.dram_tensor("st", [P, S], dt.float16, kind="ExternalInput").ap()
    ones = nc.dram_tensor("ones", [P, P], dt.float16,
                          kind="ExternalInput").ap()
    y = nc.dram_tensor("y", [S, D], dt.float16, kind="ExternalOutput").ap()

    y3 = y.rearrange("(o p) n -> p o n", p=P)        # [128, 16, 2048]

    NB = NCH * H_LOC  # 16 attention blocks, b = qc*4 + h

    with tile.TileContext(nc) as tc:
        with (
            tc.tile_pool(name="persist", bufs=1) as pp,
            tc.tile_pool(name="xcp", bufs=2) as xcp,
            tc.tile_pool(name="etp", bufs=16) as etp,
            tc.tile_pool(name="accp", bufs=3) as accp,
            tc.tile_pool(name="psc", bufs=1, space="PSUM") as psc,
        ):
            qTp = pp.tile([P, H_LOC, S], dt.float16, tag="qTp")
            kTp = pp.tile([P, H_LOC, S], dt.float16, tag="kTp")
            v_sb = pp.tile([P, NT, F], dt.float16, tag="v")
            wv_sb = pp.tile([P, DCH, F], dt.float16, tag="wv")
            wo_sb = pp.tile([P, H_LOC, D], dt.float16, tag="wo")
            ones_sb = pp.tile([P, P], dt.float16, tag="ones")

            ps_par = [0]

            def ps_alloc():
                # alternate between the two [P, CH] PSUM rings -> 4 rotating
                # accumulation groups for the projection phases
                tag = "pvps" if ps_par[0] % 2 == 0 else "pyps"
                ps_par[0] += 1
                return psc.tile([P, CH], dt.float32, tag=tag, bufs=2,
                                name=f"ps{ps_par[0]}")

            from collections import deque
            sc_iters = deque()
            acc_of = {}
            rec_of = {}

            def scores_gen(b):
                """Emit one score+exp+acc unit (2 matmuls) per yield, so
                callers can interleave units with other TensorE work."""
                qc, h = divmod(b, H_LOC)
                qsl = slice(qc * CH, (qc + 1) * CH)
                ets = []
                acc = accp.tile([P, 2, CH], dt.float16, tag="acc")
                acc_of[b] = (acc, ets)
                for ktp in range(NT // 2):
                    ss = psc.tile([P, 2, CH], dt.float32, tag="ss", bufs=2)
                    for i in range(2):
                        kt = 2 * ktp + i
                        nc.tensor.matmul(
                            ss[:, i, :], kTp[:, h, kt * P:(kt + 1) * P],
                            qTp[:, h, qsl], start=True, stop=True)
                    et = etp.tile([P, 2, CH], dt.float16, tag="et")
                    nc.scalar.activation(
                        et[:], ss[:], mybir.ActivationFunctionType.Exp)
                    if ktp == 0:
                        nc.vector.tensor_copy(acc[:], et[:])
                    else:
                        nc.vector.tensor_add(out=acc[:], in0=acc[:],
                                             in1=et[:])
                    ets.append(et)
                    yield

            def pump(n=1):
                for _ in range(n):
                    while sc_iters:
                        try:
                            next(sc_iters[0])
                            break
                        except StopIteration:
                            sc_iters.popleft()

            # ---- phase 1: k and q projections + RoPE, v proj interleaved --
            with (
                tc.tile_pool(name="wp", bufs=1) as wp,
                tc.tile_pool(name="t1p", bufs=2) as t1p,
                tc.tile_pool(name="t2p", bufs=2) as t2p,
            ):
                wk_sb = wp.tile([P, DCH, F], dt.float16, tag="wk")
                wq_sb = wp.tile([P, DCH, F], dt.float16, tag="wq")
                ct_sb = wp.tile([P, S], dt.float16, tag="ct")
                st_sb = wp.tile([P, S], dt.float16, tag="st")

                # DMA issue order = need order. Quarter slices let the first
                # matmul start after ~1MB lands instead of 4MB.
                kxc = [xcp.tile([P, DCH, CH], dt.float16, tag="xc",
                                name=f"kxc{i}") for i in range(2)]
                # leading slices are per-dc / per-dc-pair so the first
                # matmul gates on ~384KB; the rest arrive in quarters
                for dc in range(4):
                    if dc % 2 == 0:
                        wsl = slice(dc, dc + 2)
                        nc.sync.dma_start(wk_sb[:, wsl, :], wkT[:, wsl, :])
                    xsl = slice(dc, dc + 1)
                    nc.sync.dma_start(kxc[0][:, xsl, :], xTc[0, :, xsl, :])
                for qd in range(1, 4):
                    wsl = slice(qd * 4, qd * 4 + 2)
                    nc.sync.dma_start(wk_sb[:, wsl, :], wkT[:, wsl, :])
                    xsl = slice(qd * 4, (qd + 1) * 4)
                    nc.sync.dma_start(kxc[0][:, xsl, :], xTc[0, :, xsl, :])
                    wsl = slice(qd * 4 + 2, qd * 4 + 4)
                    nc.sync.dma_start(wk_sb[:, wsl, :], wkT[:, wsl, :])
                nc.sync.dma_start(ct_sb[:], ct[:])
                nc.sync.dma_start(st_sb[:], st[:])
                for qd in range(4):
                    dsl = slice(qd * 4, (qd + 1) * 4)
                    nc.sync.dma_start(kxc[1][:, dsl, :], xTc[1, :, dsl, :])
                for qd in range(4):
                    dsl = slice(qd * 4, (qd + 1) * 4)
                    nc.sync.dma_start(wq_sb[:, dsl, :], wqT[:, dsl, :])
                nc.sync.dma_start(ones_sb[:], ones[:])

                def proj_rope(w_sb, outT, nchunk, xc):
                    """One n-chunk of a q/k projection + RoPE into outT."""
                    nsl = slice(nchunk * CH, (nchunk + 1) * CH)
                    for h in range(H_LOC):
                        ps = ps_alloc()
                        for dc in range(DCH):
                            nc.tensor.matmul(
                                ps[:], w_sb[:, dc, h * HD:(h + 1) * HD],
                                xc[:, dc, :],
                                start=(dc == 0), stop=(dc == DCH - 1))
                        # RoPE: partitions 0:64 = even pairs e, 64:128 odd o:
                        #   out_e = e*c - o*s ; out_o = e*s + o*c
                        # t2 is written with its halves swapped (reading ps
                        # from PSUM allows the cross-partition access) so the
                        # sub/add below see partition-aligned SBUF inputs.
                        t1 = t1p.tile([P, CH], dt.float32, tag="t1")
                        t2 = t2p.tile([P, CH], dt.float16, tag="t2")
                        nc.vector.tensor_mul(out=t1[:], in0=ps[:],
                                             in1=ct_sb[:, nsl])
                        nc.vector.tensor_mul(out=t2[0:64, :],
                                             in0=ps[64:128, :],
                                             in1=st_sb[64:128, nsl])
                        nc.vector.tensor_mul(out=t2[64:128, :],
                                             in0=ps[0:64, :],
                                             in1=st_sb[0:64, nsl])
                        o_sl = outT[:, h, nsl]
                        nc.vector.tensor_sub(out=o_sl[0:64, :], in0=t1[0:64, :],
                                             in1=t2[0:64, :])
                        nc.vector.tensor_add(out=o_sl[64:128, :],
                                             in0=t2[64:128, :],
                                             in1=t1[64:128, :])

                def xc_load(nchunk, name):
                    t = xcp.tile([P, DCH, CH], dt.float16, tag="xc", name=name)
                    nc.sync.dma_start(t[:], xTc[nchunk])
                    return t

                # k pass: chunk n+1's tile is allocated after chunk n-1's
                # matmuls are emitted (ring bufs=2), keeping DMA one chunk
                # ahead of TensorE.
                proj_rope(wk_sb, kTp, 0, kxc[0])
                kxc.append(xc_load(2, "kxc2"))
                proj_rope(wk_sb, kTp, 1, kxc[1])
                kxc.append(xc_load(3, "kxc3"))
                proj_rope(wk_sb, kTp, 2, kxc[2])
                qxc = xc_load(0, "qxc0")
                proj_rope(wk_sb, kTp, 3, kxc[3])
                nc.sync.dma_start(wv_sb[:], wvT[:])

                # q+v pass: each chunk tile serves the q projection and the
                # v projection before the ring recycles it. scores(0) and
                # scores(1) are pumped in fine-grained units between v PSUM
                # groups of chunks 1 and 2.
                for n in range(NCH):
                    proj_rope(wq_sb, qTp, n, qxc)
                    if n == 0:
                        nc.sync.dma_start(wo_sb[:], woT[:])
                    if 1 <= n <= 2:
                        sc_iters.append(scores_gen(n - 1))
                    for nt in range(NCH):
                        ps = ps_alloc()
                        for dc in range(DCH):
                            nc.tensor.matmul(
                                ps[:], qxc[:, dc, nt * P:(nt + 1) * P],
                                wv_sb[:, dc, :],
                                start=(dc == 0), stop=(dc == DCH - 1))
                        nc.scalar.activation(
                            v_sb[:, n * NCH + nt, :], ps[:],
                            mybir.ActivationFunctionType.Copy)
                        if 1 <= n <= 2:
                            pump(2)
                    if n + 1 < NCH:
                        qxc = xc_load(n + 1, f"qxc{n + 1}")

            # ---- phase 2: pv + denominators + output projection ----------
            with (
                tc.tile_pool(name="attnp", bufs=2) as attnp,
                tc.tile_pool(name="recp", bufs=2) as recp,
                tc.tile_pool(name="ytp", bufs=4) as ytp,
            ):
                def emit_denom(b):
                    # softmax denominator: add the two acc halves on DVE,
                    # then one matmul against the all-ones stationary tile
                    # broadcasts the cross-partition sum to all partitions.
                    acc, _ = acc_of[b]
                    da = accp.tile([P, CH], dt.float16, tag="dadd", bufs=2)
                    nc.vector.tensor_add(out=da[:], in0=acc[:, 0, :],
                                         in1=acc[:, 1, :])
                    dn = psc.tile([P, CH], dt.float32, tag="pyps", bufs=2)
                    nc.tensor.matmul(dn[:], ones_sb[:], da[:],
                                     start=True, stop=True)
                    rc = recp.tile([P, CH], dt.float32, tag="rec")
                    nc.vector.reciprocal_approx_fast(rc[:], dn[:])
                    rec_of[b] = rc

                def pv_block(b, attn_cur):
                    qc, h = divmod(b, H_LOC)
                    hsl = slice(h * HD, (h + 1) * HD)
                    acc, ets = acc_of.pop(b)
                    pv = psc.tile([P, CH], dt.float32, tag="pvps", bufs=2)
                    for ktp in range(NT // 2):
                        et = ets[ktp]
                        for i in range(2):
                            kt = 2 * ktp + i
                            nc.tensor.matmul(
                                pv[:], v_sb[:, kt, hsl], et[:, i, :],
                                start=(kt == 0), stop=(kt == NT - 1))
                        pump(1)
                    # divide immediately: frees the pv PSUM slot after one
                    # DVE op instead of holding it across the denominator
                    nc.vector.tensor_mul(
                        out=attn_cur[:, h, :], in0=pv[:],
                        in1=rec_of.pop(b)[:])

                def proj_chunk(qc, attn_cur, ntls=range(NCH)):
                    for ntl in ntls:
                        nt = qc * NCH + ntl
                        for half in range(2):
                            yt = ytp.tile([P, D // 2], dt.float16, tag="yt")
                            for i in range(2):
                                oc = half * 2 + i
                                py = psc.tile([P, CH], dt.float32, tag="pyps",
                                              bufs=2)
                                for h in range(H_LOC):
                                    nc.tensor.matmul(
                                        py[:],
                                        attn_cur[:, h, ntl * P:(ntl + 1) * P],
                                        wo_sb[:, h, oc * CH:(oc + 1) * CH],
                                        start=(h == 0), stop=(h == H_LOC - 1))
                                nc.scalar.activation(
                                    yt[:, i * CH:(i + 1) * CH], py[:],
                                    mybir.ActivationFunctionType.Copy)
                                # per-half DMA: the last output store chases
                                # its copy instead of waiting for the pair
                                nc.sync.dma_start(
                                    y3[:, nt, oc * CH:(oc + 1) * CH],
                                    yt[:, i * CH:(i + 1) * CH])

                # steady state: [pv(b) | scores(b+2) units | proj(qc-1)]
                attn_hist = {}
                emit_denom(0)
                for b in range(NB):
                    qc = b // H_LOC
                    if b % H_LOC == 0:
                        attn_hist[qc] = attnp.tile([P, H_LOC, CH], dt.float16,
                                                   tag="attn",
                                                   name=f"attn_{qc}")
                    if b + 2 < NB:
                        sc_iters.append(scores_gen(b + 2))
                    pv_block(b, attn_hist[qc])
                    if b % H_LOC == 0 and b > 0:
                        proj_chunk(qc - 1, attn_hist.pop(qc - 1))
                    if b + 1 < NB:
                        emit_denom(b + 1)
                pump(100)
                proj_chunk(NCH - 1, attn_hist.pop(NCH - 1))

    nc.compile()
    return nc


_NC_CACHE = None


def _get_program():
    global _NC_CACHE
    if _NC_CACHE is None:
        _NC_CACHE = _build_program()
    return _NC_CACHE


def _rope_tables():
    scale = np.arange(0, HD, 2, dtype=np.float32) / HD
    inv_freq = 1.0 / (10000.0 ** scale)                 # [64]
    t = np.arange(S, dtype=np.float32)
    ang = np.outer(t, inv_freq)                         # [S, 64]
    cos = np.cos(ang).T.astype(np.float32)              # [64, S]
    sin = np.sin(ang).T.astype(np.float32)
    stk = lambda a: np.ascontiguousarray(
        np.concatenate([a, a], axis=0)).astype(_F16)    # [128, S]
    return stk(cos), stk(sin)


def prepare_in_maps(x, wq, wk, wv, wo):
    x = np.asarray(x, dtype=np.float32)
    wq = np.asarray(wq, dtype=np.float32) * np.float32(1.0 / np.sqrt(HD))
    wk = np.asarray(wk, dtype=np.float32)
    wv = np.asarray(wv, dtype=np.float32)
    wo = np.asarray(wo, dtype=np.float32)

    ct_t, st_t = _rope_tables()
    ones_t = np.ones((P, P), dtype=_F16)

    # even/odd RoPE permutation of rows within each head
    perm = np.concatenate([np.arange(0, HD, 2), np.arange(1, HD, 2)])

    # [NCH, P, DCH, CH]: per-partition-contiguous x chunks
    xTc = [np.ascontiguousarray(
        x[b].T.reshape(DCH, P, NCH, CH).transpose(2, 1, 0, 3)).astype(_F16)
        for b in range(B)]

    in_maps = []
    for c in range(N_CORES):
        b, hg = divmod(c, H_LOC)
        heads = np.arange(hg * H_LOC, (hg + 1) * H_LOC)
        rows_qk = (heads[:, None] * HD + perm[None, :]).reshape(-1)  # [512]
        rows_nat = np.arange(hg * F, (hg + 1) * F)
        def pmaj(wT, groups):  # [D_in, F] -> [P, groups, F]
            return np.ascontiguousarray(
                wT.reshape(groups, P, wT.shape[1]).transpose(1, 0, 2)
            ).astype(_F16)
        in_maps.append({
            "xTc": xTc[b],
            "wqT": pmaj(wq[rows_qk].T, DCH),
            "wkT": pmaj(wk[rows_qk].T, DCH),
            "wvT": pmaj(wv[rows_nat].T, DCH),
            "woT": pmaj(wo[:, rows_nat].T, H_LOC),
            "ct": ct_t, "st": st_t, "ones": ones_t,
        })
    return in_maps


def combine_results(results):
    out = np.zeros((B, S, D), dtype=np.float32)
    for c, r in enumerate(results):
        out[c // H_LOC] += r["y"].astype(np.float32)
    return out


def kernel(x, wq, wk, wv, wo):
    from concourse.bass_utils import run_bass_kernel_spmd

    nc = _get_program()
    in_maps = prepare_in_maps(x, wq, wk, wv, wo)
    res = run_bass_kernel_spmd(nc, in_maps, core_ids=list(range(N_CORES)))
    return combine_results(res.results)


if __name__ == "__main__":
    rng = np.random.default_rng(0)
    ins = {
        "x": rng.standard_normal((B, S, D), dtype=np.float32),
        "wq": rng.standard_normal((D, D), dtype=np.float32) / np.sqrt(D),
        "wk": rng.standard_normal((D, D), dtype=np.float32) / np.sqrt(D),
        "wv": rng.standard_normal((D, D), dtype=np.float32) / np.sqrt(D),
        "wo": rng.standard_normal((D, D), dtype=np.float32) / np.sqrt(D),
    }
    out = kernel(**ins)
    print("out", out.shape, out.dtype, np.abs(out).max())
